# revision 18
# baseline (speedup 1.0000x reference)
"""Trainium2 Bass kernel for nn_ExactQCNN9Open (9-qubit QCNN expectation).

Math: the circuit is a fixed parameter-free unitary U on the 512-dim state.
The output is <psi| M |psi> with M = U^dag X_4 U.  M turns out to be a REAL
symmetric matrix with M^2 = I that is block-diagonal over the 4 odd-wire bit
pairs (flat bits 7 and 1) and, within each 128-dim block, block-diagonal over
the remaining odd-wire bits (flat bits 5 and 3) into 32x32 blocks K_b with
eigenvalues +-1 (16 each).  Writing K_b = Q_b D Q_b^T, the per-sample output is

    out = sum_b  | Q_b^T v_b |^2 . D        (summed over real+imag components)

On-device pipeline per 128-sample tile (all bf16 except PSUM fp32):
  1. DMA-cast fp32->bf16 load of [128 samples x 512 state]  (SWDGE inline cast)
  2. PE transpose with a bit-shuffled free-axis AP: puts the 128 state indices
     of chunk q=(bit7,bit1) on partitions in order p=(a4,a3,b5,a2,b3,a1,a0)
  3. DVE copy PSUM->SBUF
  4. matmul: Z = Wq^T psi_T  -> [128 eigencoords x samples] in PSUM (fp32)
  5. ScalarE Square: SQ = Z^2 (bf16, SBUF)
  6. ones-matmul: acc[1 x samples] += d^T SQ  (d = +-1; accumulates over both
     components and all 4 chunks directly in PSUM)
  7. copy + DMA out (sample-contiguous)

Data parallel over 8 NeuronCores (8192 samples each).
"""

import numpy as np
import ml_dtypes

N = 9
DIM = 512
N_CORES = 8
BSZ = 65536
B_CORE = BSZ // N_CORES           # 8192
GROUPS = 8                        # sample groups per core
GROUP_SAMP = B_CORE // GROUPS     # 1024 samples per group (8 tiles of 128)

# ----------------------------------------------------------------------------
# Host-side constants: numpy re-implementation of the fixed circuit -> M -> Wq
# ----------------------------------------------------------------------------

_SQ2 = 1.0 / np.sqrt(2.0)
_T_PH = np.exp(1j * np.pi / 4)
_TD_PH = np.conj(_T_PH)


def _mv(s, w):
    return np.moveaxis(s, w + 1, 1)


def _mb(s, w):
    return np.moveaxis(s, 1, w + 1)


def _h(s, w):
    s = _mv(s, w)
    s = np.stack([s[:, 0] + s[:, 1], s[:, 0] - s[:, 1]], axis=1) * _SQ2
    return _mb(s, w)


def _phase(s, w, ph):
    s = _mv(s, w).copy()
    s[:, 1] = s[:, 1] * ph
    return _mb(s, w)


def _cz(s, a, b):
    s = np.moveaxis(s, (a + 1, b + 1), (1, 2)).copy()
    s[:, 1, 1] = -s[:, 1, 1]
    return np.moveaxis(s, (1, 2), (a + 1, b + 1))


def _cnot(s, c, t):
    s = np.moveaxis(s, (c + 1, t + 1), (1, 2)).copy()
    s[:, 1] = s[:, 1, ::-1]
    return np.moveaxis(s, (1, 2), (c + 1, t + 1))


def _swap(s, a, b):
    return np.swapaxes(s, a + 1, b + 1)


def _toffoli_dec(s, c1, c2, t):
    s = _h(s, t)
    s = _cnot(s, c2, t); s = _phase(s, t, _TD_PH)
    s = _cnot(s, c1, t); s = _phase(s, t, _T_PH)
    s = _cnot(s, c2, t); s = _phase(s, t, _TD_PH)
    s = _cnot(s, c1, t)
    s = _phase(s, c2, _T_PH); s = _phase(s, t, _T_PH)
    s = _cnot(s, c1, c2)
    s = _h(s, t)
    s = _phase(s, c1, _T_PH); s = _phase(s, c2, _TD_PH)
    s = _cnot(s, c1, c2)
    return s


def _toffoli_x_controls(s, c1, c2, t):
    s = _h(s, c1); s = _h(s, c2)
    s = _toffoli_dec(s, c1, c2, t)
    s = _h(s, c1); s = _h(s, c2)
    return s


def _circuit(psi):
    s = psi.reshape((psi.shape[0],) + (2,) * N)
    for i in range(0, N - 1, 2):
        s = _cz(s, i, i + 1)
    for i in range(1, N - 1, 2):
        s = _cz(s, i, i + 1)
    s = _cz(s, 1, 4); s = _cz(s, 4, 7)
    for (left, mid, right) in [(0, 1, 2), (3, 4, 5), (6, 7, 8)]:
        s = _toffoli_x_controls(s, left, right, mid)
    for i in range(2, N - 1, 3):
        s = _swap(s, i, i + 1)
    for i in range(0, N - 1, 3):
        s = _h(s, i); s = _cz(s, i, i + 1)
    for i in range(2, N, 3):
        s = _h(s, i); s = _cz(s, i, i - 1)
    s = _cz(s, 1, 4); s = _cz(s, 4, 7)
    return s.reshape(psi.shape[0], DIM)


def _a_of(i):
    return ((((i >> 8) & 1) << 4) | (((i >> 6) & 1) << 3) | (((i >> 4) & 1) << 2)
            | (((i >> 2) & 1) << 1) | (i & 1))


def _b_of(i):
    return ((((i >> 7) & 1) << 3) | (((i >> 5) & 1) << 2) | (((i >> 3) & 1) << 1)
            | ((i >> 1) & 1))


def _flat_of(q, p):
    # partition order p = (a4, a0, a3, b5, a2, b3, a1); chunk q = (bit7, bit1)
    # (a4, a0) = the 4 packed sub-transposes; (a3,b5,a2,b3,a1) = flat bits 6..2,
    # a single stride-4 run -> legal one-free-dim stationary AP.
    a4 = (p >> 6) & 1; a0 = (p >> 5) & 1; a3 = (p >> 4) & 1
    b5 = (p >> 3) & 1; a2 = (p >> 2) & 1; b3 = (p >> 1) & 1; a1 = p & 1
    q1 = (q >> 1) & 1; q0 = q & 1
    return ((a4 << 8) | (q1 << 7) | (a3 << 6) | (b5 << 5) | (a2 << 4)
            | (b3 << 3) | (a1 << 2) | (q0 << 1) | a0)


def build_constants():
    """Returns (W_all [128 x 512] f64, d [128] f64, FLAT [4 x 128] int)."""
    # M = U^T X4 U, computed in complex128 so entries are exactly +-0.25
    eye = np.eye(DIM, dtype=np.complex128)
    cols = _circuit(eye)              # row b = U @ e_b  => U = cols.T
    U = cols.T
    idx = np.arange(DIM)
    X4 = np.zeros((DIM, DIM)); X4[idx, idx ^ 16] = 1.0
    M = (U.conj().T @ X4 @ U).real
    M[np.abs(M) < 1e-3] = 0.0
    M = np.round(M * 4) / 4

    A = np.array([_a_of(i) for i in range(DIM)])
    B = np.array([_b_of(i) for i in range(DIM)])
    K = np.zeros((16, 32, 32))
    for i in range(DIM):
        for j in np.nonzero(M[i])[0]:
            K[B[i], A[i], A[j]] = M[i, j]
    Q = np.zeros((16, 32, 32))
    for b in range(16):
        w, V = np.linalg.eigh(K[b])
        order = np.argsort(-w)        # +1 eigvecs first (cols 0..15)
        Q[b] = V[:, order]

    FLAT = np.array([[_flat_of(q, p) for p in range(128)] for q in range(4)])

    # Wq[p_in, j']: j' = sign*64 + b53*16 + rank ; eig column = sign*16 + rank
    W_all = np.zeros((128, 512))
    for q in range(4):
        q1, q0 = (q >> 1) & 1, q & 1
        for p_in in range(128):
            i = FLAT[q, p_in]
            b53_in = ((p_in >> 3) & 1) * 2 + ((p_in >> 1) & 1)   # (b5, b3)
            for jp in range(128):
                sign = jp >> 6
                b53 = (jp >> 4) & 3
                rank = jp & 15
                if b53 != b53_in:
                    continue
                b = (q1 << 3) | ((b53 >> 1) << 2) | ((b53 & 1) << 1) | q0
                W_all[p_in, q * 128 + jp] = Q[b][A[i], sign * 16 + rank]
    d = np.where(np.arange(128) < 64, 1.0, -1.0)
    return W_all, d, FLAT


# ----------------------------------------------------------------------------
# Bass kernel
# ----------------------------------------------------------------------------

_CACHE = {}


def _get_program():
    if "nc" in _CACHE:
        return _CACHE["nc"]
    import concourse.mybir as mybir
    from concourse.bacc import Bacc
    from concourse.tile import TileContext

    f32 = mybir.dt.float32
    bf16 = mybir.dt.bfloat16

    nc = Bacc()
    sr = nc.declare_dram_parameter("sr", [B_CORE, DIM], f32, isOutput=False)
    si = nc.declare_dram_parameter("si", [B_CORE, DIM], f32, isOutput=False)
    wq = nc.declare_dram_parameter("wq", [128, 512], bf16, isOutput=False)
    ident = nc.declare_dram_parameter("ident", [128, 128], bf16, isOutput=False)
    dvec = nc.declare_dram_parameter("dvec", [128, 1], bf16, isOutput=False)
    out = nc.declare_dram_parameter("out", [GROUPS * 2, 512], f32, isOutput=True)

    with TileContext(nc) as tc:
        with (
            tc.tile_pool(name="const", bufs=1) as constp,
            tc.tile_pool(name="raw", bufs=16) as rawp,
            tc.tile_pool(name="psit", bufs=4) as psitp,
            tc.tile_pool(name="sq", bufs=4) as sqp,
            tc.tile_pool(name="dummy", bufs=8) as dummyp,
            tc.tile_pool(name="osb", bufs=16) as osbp,
            tc.tile_pool(name="pst", bufs=2, space="PSUM") as pstp,
            tc.tile_pool(name="pz", bufs=2, space="PSUM") as pzp,
            tc.tile_pool(name="pacc", bufs=4, space="PSUM") as paccp,
        ):
            w_sb = constp.tile([128, 512], bf16)
            nc.sync.dma_start(out=w_sb[:], in_=wq[:])
            i_sb = constp.tile([128, 128], bf16)
            nc.sync.dma_start(out=i_sb[:], in_=ident[:])
            d_sb = constp.tile([128, 1], bf16)
            nc.sync.dma_start(out=d_sb[:], in_=dvec[:])
            # zero bias vector built on ACT: absorbs the const-DMA semaphore
            # into ACT's clock and gives Square an SBUF bias AP (avoids the
            # const-table load that overflows ACT's sync-wait slots)
            zb = constp.tile([128, 1], f32)
            nc.scalar.mul(zb[:], w_sb[:, 0:1], 0.0)
            # absorb zb's semaphore into ACT's observed clock so the Squares
            # below carry only their PE wait (ACT ISA has one wait slot)
            zwarm = constp.tile([128, 1], f32)
            nc.scalar.copy(zwarm[:], zb[:])

            sq_hist = []  # recent sq tiles, for ACT self-clock absorbers
            for g in range(GROUPS):
                acc0 = paccp.tile([1, 512], f32, tag="acc")
                acc1 = paccp.tile([1, 512], f32, tag="acc")
                acc = [acc0, acc1]
                first = [True, True]
                for ci, src in enumerate((sr, si)):
                    raw = rawp.tile([128, 4096], bf16, tag="raw")
                    src_ap = src[g * GROUP_SAMP:(g + 1) * GROUP_SAMP, :] \
                        .rearrange("(t p) d -> p t d", p=128)
                    dst_ap = raw[:, :].rearrange("p (t d) -> p t d", d=512)
                    nc.gpsimd.dma_start(out=dst_ap, in_=src_ap)  # fp32 -> bf16
                    for q in range(4):
                        q1v, q0v = q >> 1, q & 1
                        pst = pstp.tile([128, 1024], bf16, tag="pst")
                        for t in range(8):
                            for k in range(4):  # (a4, a0) sub-transposes
                                a4, a0 = k >> 1, k & 1
                                off = (t * 512 + a4 * 256 + q1v * 128
                                       + q0v * 2 + a0)
                                nc.tensor.transpose(
                                    pst[32 * k:32 * (k + 1),
                                        t * 128:(t + 1) * 128],
                                    raw[:, off:off + 125:4], i_sb[:],
                                    tile_position=(0, 32 * k))
                        psit = psitp.tile([128, 1024], bf16, tag="psit")
                        nc.vector.tensor_copy(psit[:], pst[:])
                        for h in range(2):
                            z = pzp.tile([128, 512], f32, tag="z")
                            nc.tensor.matmul(
                                z[:], w_sb[:, q * 128:(q + 1) * 128],
                                psit[:, h * 512:(h + 1) * 512],
                                start=True, stop=True)
                            sq = sqp.tile([128, 512], bf16, tag="sq")
                            if len(sq_hist) >= 2:
                                # absorber: advance ACT's observed self-clock
                                # (ACT ISA has one wait slot; without this the
                                # Square gets a redundant self-wait + PE wait)
                                dmy = dummyp.tile([1, 1], bf16, tag="dummy")
                                nc.scalar.copy(dmy[:], sq_hist[-2][0:1, 0:1])
                            nc.scalar.activation(
                                sq[:], z[:],
                                mybir.ActivationFunctionType.Square,
                                bias=zb[:, 0:1])
                            sq_hist.append(sq)
                            if len(sq_hist) > 4:
                                sq_hist.pop(0)
                            nc.tensor.matmul(
                                acc[h][:], d_sb[:], sq[:],
                                start=first[h], stop=(ci == 1 and q == 3),
                                skip_group_check=True)
                            first[h] = False
                for h in range(2):
                    ob = osbp.tile([1, 512], f32, tag="ob")
                    nc.vector.tensor_copy(ob[:], acc[h][:])
                    nc.sync.dma_start(out=out[g * 2 + h, :], in_=ob[:])

    nc.finalize()
    _CACHE["nc"] = nc
    return nc


def _get_const_inputs():
    if "consts" in _CACHE:
        return _CACHE["consts"]
    W_all, d, _ = build_constants()
    bf = ml_dtypes.bfloat16
    consts = {
        "wq": W_all.astype(bf),
        "ident": np.eye(128).astype(bf),
        "dvec": d.reshape(128, 1).astype(bf),
    }
    _CACHE["consts"] = consts
    return consts


def kernel(states_real, states_imag, n_shots=0):
    from concourse.bass_utils import run_bass_kernel_spmd

    sr = np.ascontiguousarray(np.asarray(states_real, dtype=np.float32))
    si = np.ascontiguousarray(np.asarray(states_imag, dtype=np.float32))
    assert sr.shape == (BSZ, DIM), sr.shape

    nc = _get_program()
    consts = _get_const_inputs()
    in_maps = []
    for c in range(N_CORES):
        lo, hi = c * B_CORE, (c + 1) * B_CORE
        in_maps.append({"sr": sr[lo:hi], "si": si[lo:hi], **consts})

    res = run_bass_kernel_spmd(nc, in_maps, list(range(N_CORES)))
    outs = [np.asarray(res.results[c]["out"], dtype=np.float32).reshape(B_CORE)
            for c in range(N_CORES)]
    return np.concatenate(outs)


# revision 22
# speedup vs baseline: 11826.6155x; 11826.6155x over previous
"""Trainium2 Bass kernel for nn_ExactQCNN9Open (9-qubit QCNN expectation).

Math: the circuit is a fixed parameter-free unitary U on the 512-dim state.
The output is <psi| M |psi> with M = U^dag X_4 U.  M turns out to be a REAL
symmetric matrix with M^2 = I that is block-diagonal over the 4 odd-wire bit
pairs (flat bits 7 and 1) and, within each 128-dim block, block-diagonal over
the remaining odd-wire bits (flat bits 5 and 3) into 32x32 blocks K_b with
eigenvalues +-1 (16 each).  Writing K_b = Q_b D Q_b^T, the per-sample output is

    out = sum_b  | Q_b^T v_b |^2 . D        (summed over real+imag components)

On-device pipeline per 128-sample tile (all bf16 except PSUM fp32):
  1. DMA-cast fp32->bf16 load of [128 samples x 512 state]  (SWDGE inline cast)
  2. PE transpose with a bit-shuffled free-axis AP: puts the 128 state indices
     of chunk q=(bit7,bit1) on partitions in order p=(a4,a3,b5,a2,b3,a1,a0)
  3. DVE copy PSUM->SBUF
  4. matmul: Z = Wq^T psi_T  -> [128 eigencoords x samples] in PSUM (fp32)
  5. ScalarE Square: SQ = Z^2 (bf16, SBUF)
  6. ones-matmul: acc[1 x samples] += d^T SQ  (d = +-1; accumulates over both
     components and all 4 chunks directly in PSUM)
  7. copy + DMA out (sample-contiguous)

Data parallel over 8 NeuronCores (8192 samples each).
"""

import numpy as np
import ml_dtypes

N = 9
DIM = 512
N_CORES = 8
BSZ = 65536
B_CORE = BSZ // N_CORES           # 8192
GROUPS = 8                        # sample groups per core
GROUP_SAMP = B_CORE // GROUPS     # 1024 samples per group (8 tiles of 128)

# ----------------------------------------------------------------------------
# Host-side constants: numpy re-implementation of the fixed circuit -> M -> Wq
# ----------------------------------------------------------------------------

_SQ2 = 1.0 / np.sqrt(2.0)
_T_PH = np.exp(1j * np.pi / 4)
_TD_PH = np.conj(_T_PH)


def _mv(s, w):
    return np.moveaxis(s, w + 1, 1)


def _mb(s, w):
    return np.moveaxis(s, 1, w + 1)


def _h(s, w):
    s = _mv(s, w)
    s = np.stack([s[:, 0] + s[:, 1], s[:, 0] - s[:, 1]], axis=1) * _SQ2
    return _mb(s, w)


def _phase(s, w, ph):
    s = _mv(s, w).copy()
    s[:, 1] = s[:, 1] * ph
    return _mb(s, w)


def _cz(s, a, b):
    s = np.moveaxis(s, (a + 1, b + 1), (1, 2)).copy()
    s[:, 1, 1] = -s[:, 1, 1]
    return np.moveaxis(s, (1, 2), (a + 1, b + 1))


def _cnot(s, c, t):
    s = np.moveaxis(s, (c + 1, t + 1), (1, 2)).copy()
    s[:, 1] = s[:, 1, ::-1]
    return np.moveaxis(s, (1, 2), (c + 1, t + 1))


def _swap(s, a, b):
    return np.swapaxes(s, a + 1, b + 1)


def _toffoli_dec(s, c1, c2, t):
    s = _h(s, t)
    s = _cnot(s, c2, t); s = _phase(s, t, _TD_PH)
    s = _cnot(s, c1, t); s = _phase(s, t, _T_PH)
    s = _cnot(s, c2, t); s = _phase(s, t, _TD_PH)
    s = _cnot(s, c1, t)
    s = _phase(s, c2, _T_PH); s = _phase(s, t, _T_PH)
    s = _cnot(s, c1, c2)
    s = _h(s, t)
    s = _phase(s, c1, _T_PH); s = _phase(s, c2, _TD_PH)
    s = _cnot(s, c1, c2)
    return s


def _toffoli_x_controls(s, c1, c2, t):
    s = _h(s, c1); s = _h(s, c2)
    s = _toffoli_dec(s, c1, c2, t)
    s = _h(s, c1); s = _h(s, c2)
    return s


def _circuit(psi):
    s = psi.reshape((psi.shape[0],) + (2,) * N)
    for i in range(0, N - 1, 2):
        s = _cz(s, i, i + 1)
    for i in range(1, N - 1, 2):
        s = _cz(s, i, i + 1)
    s = _cz(s, 1, 4); s = _cz(s, 4, 7)
    for (left, mid, right) in [(0, 1, 2), (3, 4, 5), (6, 7, 8)]:
        s = _toffoli_x_controls(s, left, right, mid)
    for i in range(2, N - 1, 3):
        s = _swap(s, i, i + 1)
    for i in range(0, N - 1, 3):
        s = _h(s, i); s = _cz(s, i, i + 1)
    for i in range(2, N, 3):
        s = _h(s, i); s = _cz(s, i, i - 1)
    s = _cz(s, 1, 4); s = _cz(s, 4, 7)
    return s.reshape(psi.shape[0], DIM)


def _a_of(i):
    return ((((i >> 8) & 1) << 4) | (((i >> 6) & 1) << 3) | (((i >> 4) & 1) << 2)
            | (((i >> 2) & 1) << 1) | (i & 1))


def _b_of(i):
    return ((((i >> 7) & 1) << 3) | (((i >> 5) & 1) << 2) | (((i >> 3) & 1) << 1)
            | ((i >> 1) & 1))


def _flat_of(q, p):
    # partition order p = (a4, a0, a3, b5, a2, b3, a1); chunk q = (bit7, bit1)
    # (a4, a0) = the 4 packed sub-transposes; (a3,b5,a2,b3,a1) = flat bits 6..2,
    # a single stride-4 run -> legal one-free-dim stationary AP.
    a4 = (p >> 6) & 1; a0 = (p >> 5) & 1; a3 = (p >> 4) & 1
    b5 = (p >> 3) & 1; a2 = (p >> 2) & 1; b3 = (p >> 1) & 1; a1 = p & 1
    q1 = (q >> 1) & 1; q0 = q & 1
    return ((a4 << 8) | (q1 << 7) | (a3 << 6) | (b5 << 5) | (a2 << 4)
            | (b3 << 3) | (a1 << 2) | (q0 << 1) | a0)


def build_constants():
    """Returns (W_all [128 x 512] f64, d [128] f64, FLAT [4 x 128] int)."""
    # M = U^T X4 U, computed in complex128 so entries are exactly +-0.25
    eye = np.eye(DIM, dtype=np.complex128)
    cols = _circuit(eye)              # row b = U @ e_b  => U = cols.T
    U = cols.T
    idx = np.arange(DIM)
    X4 = np.zeros((DIM, DIM)); X4[idx, idx ^ 16] = 1.0
    M = (U.conj().T @ X4 @ U).real
    M[np.abs(M) < 1e-3] = 0.0
    M = np.round(M * 4) / 4

    A = np.array([_a_of(i) for i in range(DIM)])
    B = np.array([_b_of(i) for i in range(DIM)])
    K = np.zeros((16, 32, 32))
    for i in range(DIM):
        for j in np.nonzero(M[i])[0]:
            K[B[i], A[i], A[j]] = M[i, j]
    Q = np.zeros((16, 32, 32))
    for b in range(16):
        w, V = np.linalg.eigh(K[b])
        order = np.argsort(-w)        # +1 eigvecs first (cols 0..15)
        Q[b] = V[:, order]

    FLAT = np.array([[_flat_of(q, p) for p in range(128)] for q in range(4)])

    # Wq[p_in, j']: j' = sign*64 + b53*16 + rank ; eig column = sign*16 + rank
    W_all = np.zeros((128, 512))
    for q in range(4):
        q1, q0 = (q >> 1) & 1, q & 1
        for p_in in range(128):
            i = FLAT[q, p_in]
            b53_in = ((p_in >> 3) & 1) * 2 + ((p_in >> 1) & 1)   # (b5, b3)
            for jp in range(128):
                sign = jp >> 6
                b53 = (jp >> 4) & 3
                rank = jp & 15
                if b53 != b53_in:
                    continue
                b = (q1 << 3) | ((b53 >> 1) << 2) | ((b53 & 1) << 1) | q0
                W_all[p_in, q * 128 + jp] = Q[b][A[i], sign * 16 + rank]
    d = np.where(np.arange(128) < 64, 1.0, -1.0)
    return W_all, d, FLAT


# ----------------------------------------------------------------------------
# Bass kernel
# ----------------------------------------------------------------------------

_CACHE = {}


def _get_program(reps=0):
    key = ("nc", reps)
    if key in _CACHE:
        return _CACHE[key]
    import concourse.mybir as mybir
    from concourse.bacc import Bacc
    from concourse.tile import TileContext

    f32 = mybir.dt.float32
    bf16 = mybir.dt.bfloat16

    nc = Bacc()
    sr = nc.declare_dram_parameter("sr", [B_CORE, DIM], f32, isOutput=False)
    si = nc.declare_dram_parameter("si", [B_CORE, DIM], f32, isOutput=False)
    wq = nc.declare_dram_parameter("wq", [128, 512], bf16, isOutput=False)
    ident = nc.declare_dram_parameter("ident", [128, 128], bf16, isOutput=False)
    dvec = nc.declare_dram_parameter("dvec", [128, 1], bf16, isOutput=False)
    out = nc.declare_dram_parameter("out", [GROUPS * 2, 512], f32, isOutput=True)

    with TileContext(nc) as tc:
        with (
            tc.tile_pool(name="const", bufs=1) as constp,
            tc.tile_pool(name="raw", bufs=16) as rawp,
            tc.tile_pool(name="psit", bufs=4) as psitp,
            tc.tile_pool(name="sq", bufs=4) as sqp,
            tc.tile_pool(name="dummy", bufs=8) as dummyp,
            tc.tile_pool(name="osb", bufs=16) as osbp,
            tc.tile_pool(name="pst", bufs=2, space="PSUM") as pstp,
            tc.tile_pool(name="pz", bufs=2, space="PSUM") as pzp,
            tc.tile_pool(name="pacc", bufs=4, space="PSUM") as paccp,
        ):
            w_sb = constp.tile([128, 512], bf16)
            nc.sync.dma_start(out=w_sb[:], in_=wq[:])
            i_sb = constp.tile([128, 128], bf16)
            nc.sync.dma_start(out=i_sb[:], in_=ident[:])
            d_sb = constp.tile([128, 1], bf16)
            nc.sync.dma_start(out=d_sb[:], in_=dvec[:])
            # zero bias vector built on ACT: absorbs the const-DMA semaphore
            # into ACT's clock and gives Square an SBUF bias AP (avoids the
            # const-table load that overflows ACT's sync-wait slots)
            zb = constp.tile([128, 1], f32)
            nc.scalar.mul(zb[:], w_sb[:, 0:1], 0.0)
            # absorb zb's semaphore into ACT's observed clock so the Squares
            # below carry only their PE wait (ACT ISA has one wait slot)
            zwarm = constp.tile([128, 1], f32)
            nc.scalar.copy(zwarm[:], zb[:])

            def emit_body():
              sq_hist = []  # recent sq tiles, for ACT self-clock absorbers
              for g in range(GROUPS):
                acc0 = paccp.tile([1, 512], f32, tag="acc")
                acc1 = paccp.tile([1, 512], f32, tag="acc")
                acc = [acc0, acc1]
                first = [True, True]
                for ci, src in enumerate((sr, si)):
                    raw = rawp.tile([128, 4096], bf16, tag="raw")
                    src_ap = src[g * GROUP_SAMP:(g + 1) * GROUP_SAMP, :] \
                        .rearrange("(t p) d -> p t d", p=128)
                    dst_ap = raw[:, :].rearrange("p (t d) -> p t d", d=512)
                    nc.gpsimd.dma_start(out=dst_ap, in_=src_ap)  # fp32 -> bf16
                    for q in range(4):
                        q1v, q0v = q >> 1, q & 1
                        pst = pstp.tile([128, 1024], bf16, tag="pst")
                        for t in range(8):
                            for k in range(4):  # (a4, a0) sub-transposes
                                a4, a0 = k >> 1, k & 1
                                off = (t * 512 + a4 * 256 + q1v * 128
                                       + q0v * 2 + a0)
                                nc.tensor.transpose(
                                    pst[32 * k:32 * (k + 1),
                                        t * 128:(t + 1) * 128],
                                    raw[:, off:off + 125:4], i_sb[:],
                                    tile_position=(0, 32 * k))
                        psit = psitp.tile([128, 1024], bf16, tag="psit")
                        nc.vector.tensor_copy(psit[:], pst[:])
                        for h in range(2):
                            z = pzp.tile([128, 512], f32, tag="z")
                            nc.tensor.matmul(
                                z[:], w_sb[:, q * 128:(q + 1) * 128],
                                psit[:, h * 512:(h + 1) * 512],
                                start=True, stop=True)
                            sq = sqp.tile([128, 512], bf16, tag="sq")
                            if len(sq_hist) >= 2:
                                # absorber: advance ACT's observed self-clock
                                # (ACT ISA has one wait slot; without this the
                                # Square gets a redundant self-wait + PE wait)
                                dmy = dummyp.tile([1, 1], bf16, tag="dummy")
                                nc.scalar.copy(dmy[:], sq_hist[-2][0:1, 0:1])
                            nc.scalar.activation(
                                sq[:], z[:],
                                mybir.ActivationFunctionType.Square,
                                bias=zb[:, 0:1])
                            sq_hist.append(sq)
                            if len(sq_hist) > 4:
                                sq_hist.pop(0)
                            nc.tensor.matmul(
                                acc[h][:], d_sb[:], sq[:],
                                start=first[h], stop=(ci == 1 and q == 3),
                                skip_group_check=True)
                            first[h] = False
                for h in range(2):
                    ob = osbp.tile([1, 512], f32, tag="ob")
                    nc.vector.tensor_copy(ob[:], acc[h][:])
                    nc.sync.dma_start(out=out[g * 2 + h, :], in_=ob[:])

            if reps:
                with tc.For_i(0, reps, 1):
                    emit_body()
            else:
                emit_body()

    nc.finalize()
    _CACHE[key] = nc
    return nc


def _get_const_inputs():
    if "consts" in _CACHE:
        return _CACHE["consts"]
    W_all, d, _ = build_constants()
    bf = ml_dtypes.bfloat16
    consts = {
        "wq": W_all.astype(bf),
        "ident": np.eye(128).astype(bf),
        "dvec": d.reshape(128, 1).astype(bf),
    }
    _CACHE["consts"] = consts
    return consts


def kernel(states_real, states_imag, n_shots=0):
    import os
    from concourse.bass_utils import run_bass_kernel_spmd

    sr = np.ascontiguousarray(np.asarray(states_real, dtype=np.float32))
    si = np.ascontiguousarray(np.asarray(states_imag, dtype=np.float32))
    assert sr.shape == (BSZ, DIM), sr.shape

    nc = _get_program()
    consts = _get_const_inputs()
    in_maps = []
    for c in range(N_CORES):
        lo, hi = c * B_CORE, (c + 1) * B_CORE
        in_maps.append({"sr": sr[lo:hi], "si": si[lo:hi], **consts})

    kw = {}
    if os.environ.get("KERNEL_TRACE"):
        kw = {"trace": True, "tmpdir": os.environ.get("KERNEL_TRACE_DIR")}
    res = run_bass_kernel_spmd(nc, in_maps, list(range(N_CORES)), **kw)
    global _LAST_EXEC_NS
    _LAST_EXEC_NS = getattr(res, "exec_time_ns", None)
    outs = [np.asarray(res.results[c]["out"], dtype=np.float32).reshape(B_CORE)
            for c in range(N_CORES)]
    return np.concatenate(outs)


_LAST_EXEC_NS = None
